# revision 8
# baseline (speedup 1.0000x reference)
"""DPQ joint classification loss on 8 Trainium2 NeuronCores.

reference math (B=4096, D=512, C=10000):
    soft_pred = soft_x @ weight.T ; hard_pred = hard_x @ weight.T
    loss = CE(soft_pred, t) + CE(hard_pred, t)
           + 0.5 * 0.5*(||soft_x - centers[t]||^2 + ||hard_x - centers[t]||^2) / B

Sharding: data-parallel over batch. Core i gets soft rows [i*512,(i+1)*512)
and the matching hard rows, stacked into X = [1024, 512]; weight/centers are
replicated. Each core returns one scalar:
    sum_rows( logsumexp(X @ W^T) - logit_at_target + 0.25*||X - centers[t]||^2 )
and the host computes loss = sum(cores) / B.

Per-core pipeline (engine balance: ACT ~70us is the pace-setter, PE fp8
~69us, DVE ~40us, Pool runs the SWDGE gathers):
  - PE: fp8(e4m3) DoubleRow GEMM at 2x rate: both operands are packed
    [128p, 2, free] so one matmul contracts 256 of the 512 k-dim. x and w
    are pre-scaled by 16 on the host (keeps w out of the fp8 subnormal
    range); the exp undoes the 256x logit scale via its input scale.
  - ACT: exp straight out of PSUM into a discarded bf16 tile (no
    max-subtraction: logits are ~N(0, 0.31), exp is safe in fp32). No
    accum_out: the 283ns/instr accumulator read would make ACT the
    bottleneck, so the row-sums ride on DVE instead.
  - DVE: per-tile row-sum of the bf16 exp tile via tensor_scalar+accum_out
    (4x mode), plus the exact bf16 target-logit (rowsum(x * w_gather)) and
    quantization (rowsum((x - c_gather)^2)) terms and the final combine.
  - GPSIMD: indirect-DMA row gathers weight[targets], centers[targets]
    from bf16 copies of the tables.
  - PE again: cross-partition sum via ones-matmul; DMA scalar out.
"""

import json

import numpy as np

B_FULL = 4096
D = 512
C = 10000
N_CORES = 8
BS = B_FULL // N_CORES          # 512 rows per core per tensor
B = 2 * BS                      # 1024 stacked rows per core
P = 128
NB = B // P                     # 8 row chunks
NKP = 2                         # k-pairs: 512 = 2 * (2*128)
GW = 2048                       # class-group width = 4 PSUM banks
PARAM = 0.5
FP8_SCALE = 16.0                # per-operand pre-scale before e4m3 cast


def _patch_bir_bytes(b: bytes, max_waits: int = 1) -> bytes:
    """Adapt Tile-emitted BIR to this walrus build: it supports only one
    sync-wait per instruction (excess waits move to preceding NoOps) and
    rejects the EVENT_SEMAPHORE_RANGE_CLEAR raw-ISA encoding (replaced by
    per-semaphore write-0 EventSemaphore ops)."""
    d = json.loads(b)
    for f in d["functions"]:
        for blk in f["blocks"]:
            new_insts = []
            for ins in blk["instructions"]:
                if (
                    ins.get("opcode") == "ISA"
                    and ins.get("op_name") == "EVENT_SEMAPHORE_RANGE_CLEAR"
                ):
                    ad = ins.get("ant_dict") or {}
                    for sem_id in range(ad["range_first"], ad["range_last"] + 1):
                        new_insts.append({
                            "name": f"{ins['name']}_clr{sem_id}",
                            "opcode": "EventSemaphore",
                            "engine": ins["engine"],
                            "ins": [],
                            "outs": [],
                            "debug": ins.get("debug"),
                            "sync_info": {
                                "on_wait": [],
                                "on_update": [{
                                    "ant_name": f"semclr_{sem_id}",
                                    "id": sem_id,
                                    "sync_type": "semaphore",
                                    "update_mode": "sem-wr-imm",
                                    "update_value": 0,
                                }],
                            },
                        })
                    continue
                si = ins.get("sync_info")
                waits = (si or {}).get("on_wait") or []
                if len(waits) > max_waits:
                    extra, keep = waits[:-max_waits], waits[-max_waits:]
                    idx = 0
                    while extra:
                        chunk, extra = extra[:max_waits], extra[max_waits:]
                        new_insts.append({
                            "name": f"{ins['name']}_w{idx}",
                            "opcode": "NoOp",
                            "engine": ins["engine"],
                            "ins": [],
                            "outs": [],
                            "debug": ins.get("debug"),
                            "sync_info": {"on_wait": chunk, "on_update": []},
                        })
                        idx += 1
                    si["on_wait"] = keep
                new_insts.append(ins)
            blk["instructions"] = new_insts
    return json.dumps(d).encode()


def _build_bass():
    import concourse.bass as bass
    import concourse.tile as tile
    from concourse import mybir

    f32 = mybir.dt.float32
    bf16 = mybir.dt.bfloat16
    f8 = mybir.dt.float8e4
    i32 = mybir.dt.int32
    AF = mybir.ActivationFunctionType
    OP = mybir.AluOpType
    DR = mybir.MatmulPerfMode.DoubleRow

    # first groups are narrow so the first psum bank (and ACT) fires while
    # the bulk of the weight stream is still in flight
    widths = [512, 512, 1024, 2048, 2048, 2048, 1808]
    assert sum(widths) == C
    groups = []
    c0 = 0
    for w in widths:
        groups.append((c0, w))
        c0 += w
    NG = len(groups)

    nc = bass.Bass()
    # fp8 DoubleRow operands, packed [128p, j=2, free]; element [p, j, n]
    # holds the k-index kp*256 + j*128 + p.
    xt_d = [nc.dram_tensor(f"xt{kp}", [P, 2, B], f8, kind="ExternalInput")
            for kp in range(NKP)]
    wt_d = [nc.dram_tensor(f"wt{kp}", [P, 2, C], f8, kind="ExternalInput")
            for kp in range(NKP)]
    x_d = nc.dram_tensor("x", [B, D], bf16, kind="ExternalInput")
    w_d = nc.dram_tensor("w", [C, D], bf16, kind="ExternalInput")
    cen_d = nc.dram_tensor("cen", [C, D], bf16, kind="ExternalInput")
    tgt_d = nc.dram_tensor("tgt", [BS, 1], i32, kind="ExternalInput")
    out_d = nc.dram_tensor("out", [1, 1], f32, kind="ExternalOutput")

    with tile.TileContext(nc) as tc:
        with (
            tc.tile_pool(name="persist", bufs=1) as persist,
            tc.tile_pool(name="wtp", bufs=2) as wtp,
            tc.tile_pool(name="scratch", bufs=3) as scratch,
        ):
            # ---- resident loads ----
            # xt feeds the matmuls: issue on the ACT ring (idle at startup)
            # so the sync ring's first trigger is already the first wt group.
            xt_sb = []
            for kp in range(NKP):
                t = persist.tile([P, 2, B], f8, tag=f"xt{kp}", name=f"xt{kp}")
                nc.scalar.dma_start(t[:, :, :], xt_d[kp][:, :, :])
                xt_sb.append(t)
            # x/tgt feed only the (small) DVE aux path; gpsimd SWDGE keeps
            # them off the HWDGE queues that stream wt.
            tgt_sb = []
            for c in range(BS // P):
                t = persist.tile([P, 1], i32, tag=f"tgt{c}", name=f"tgt{c}")
                nc.gpsimd.dma_start(t[:, :], tgt_d[c * P:(c + 1) * P, :])
                tgt_sb.append(t)
            x_sb = []
            for b in range(NB):
                t = persist.tile([P, D], bf16, tag=f"x{b}", name=f"x{b}")
                nc.gpsimd.dma_start(t[:, :], x_d[b * P:(b + 1) * P, :])
                x_sb.append(t)

            # ---- gathers: weight[targets], centers[targets] (bf16) ----
            wg_sb, cg_sb = [], []
            for c in range(BS // P):
                wg = persist.tile([P, D], bf16, tag=f"wg{c}", name=f"wg{c}")
                nc.gpsimd.indirect_dma_start(
                    out=wg[:, :], out_offset=None, in_=w_d[:, :],
                    in_offset=bass.IndirectOffsetOnAxis(ap=tgt_sb[c][:, :1], axis=0),
                )
                wg_sb.append(wg)
                cg = persist.tile([P, D], bf16, tag=f"cg{c}", name=f"cg{c}")
                nc.gpsimd.indirect_dma_start(
                    out=cg[:, :], out_offset=None, in_=cen_d[:, :],
                    in_offset=bass.IndirectOffsetOnAxis(ap=tgt_sb[c][:, :1], axis=0),
                )
                cg_sb.append(cg)

            # ---- small result tiles ----
            sums = persist.tile([P, NB, NG], f32, name="sums")
            se = persist.tile([P, NB], f32, name="se")
            lse = persist.tile([P, NB], f32, name="lse")
            tcol = persist.tile([P, NB], f32, name="tcol")
            qcol = persist.tile([P, NB], f32, name="qcol")
            ctr1 = persist.tile([P, NB], f32, name="ctr1")
            ctr2 = persist.tile([P, NB], f32, name="ctr2")
            rowtot = persist.tile([P, 1], f32, name="rowtot")
            ones = persist.tile([P, 1], f32, name="ones")
            nc.vector.memset(ones[:, :], 1.0)

            # ---- aux path on DVE (all bf16 for the 2x/4x modes): target
            # logits + quantization ----
            for b in range(NB):
                c = b % (BS // P)
                pr = scratch.tile([P, D], bf16, tag="pr", name=f"pr{b}")
                nc.vector.tensor_mul(pr[:, :], x_sb[b][:, :], wg_sb[c][:, :])
                dm0 = scratch.tile([P, D], bf16, tag="dm0", name=f"dm0_{b}")
                nc.vector.tensor_scalar(
                    out=dm0[:, :], in0=pr[:, :], scalar1=1.0, scalar2=0.0,
                    op0=OP.mult, op1=OP.add, accum_out=tcol[:, b:b + 1],
                )
                df = scratch.tile([P, D], bf16, tag="df", name=f"df{b}")
                nc.vector.tensor_sub(df[:, :], x_sb[b][:, :], cg_sb[c][:, :])
                sq = scratch.tile([P, D], bf16, tag="sq", name=f"sq{b}")
                nc.vector.tensor_mul(sq[:, :], df[:, :], df[:, :])
                dm1 = scratch.tile([P, D], bf16, tag="dm1", name=f"dm1_{b}")
                nc.vector.tensor_scalar(
                    out=dm1[:, :], in0=sq[:, :], scalar1=1.0, scalar2=0.0,
                    op0=OP.mult, op1=OP.add, accum_out=qcol[:, b:b + 1],
                )

            # ---- main GEMM (fp8 DoubleRow) + exp, row-sums on DVE ----
            with tc.tile_pool(name="psum", bufs=2, space="PSUM") as psum_pool:
                for g, (c0, cw) in enumerate(groups):
                    wt_g = []
                    for kp in range(NKP):
                        t = wtp.tile([P, 2, cw], f8, tag=f"wt{kp}", name=f"wt{kp}_{g}")
                        nc.sync.dma_start(t[:, :, :cw], wt_d[kp][:, :, c0:c0 + cw])
                        wt_g.append(t)
                    for b in range(NB):
                        ps = psum_pool.tile([P, cw], f32, tag="ps", name=f"ps{g}_{b}")
                        nbank = (cw + 511) // 512
                        for bank in range(nbank):
                            s0 = bank * 512
                            sw = min(512, cw - s0)
                            for kp in range(NKP):
                                nc.tensor.matmul(
                                    ps[:, s0:s0 + sw],
                                    lhsT=xt_sb[kp][:, :, b * P:(b + 1) * P],
                                    rhs=wt_g[kp][:, :, s0:s0 + sw],
                                    start=(kp == 0), stop=(kp == NKP - 1),
                                    perf_mode=DR,
                                )
                        es = scratch.tile([P, cw], f32, tag="es", name=f"es{g}_{b}")
                        nc.scalar.activation(
                            es[:, :cw], ps[:, :cw], AF.Exp,
                            scale=1.0 / (FP8_SCALE * FP8_SCALE),
                            accum_out=sums[:, b, g:g + 1],
                        )

            # ---- logsumexp + per-row combine ----
            nc.vector.tensor_reduce(
                out=se[:, :], in_=sums[:, :, :],
                axis=mybir.AxisListType.X, op=OP.add,
            )
            nc.scalar.activation(lse[:, :], se[:, :], AF.Ln)
            nc.vector.tensor_sub(ctr1[:, :], lse[:, :], tcol[:, :])
            nc.vector.scalar_tensor_tensor(
                out=ctr2[:, :], in0=qcol[:, :], scalar=0.25, in1=ctr1[:, :],
                op0=OP.mult, op1=OP.add,
            )
            nc.vector.tensor_reduce(
                out=rowtot[:, :], in_=ctr2[:, :],
                axis=mybir.AxisListType.X, op=OP.add,
            )

            # ---- cross-partition sum via ones-matmul, write scalar ----
            with tc.tile_pool(name="psum2", bufs=1, space="PSUM") as pp2:
                tot_ps = pp2.tile([1, 1], f32, name="tot_ps")
                nc.tensor.matmul(
                    tot_ps[:, :], lhsT=rowtot[:, :], rhs=ones[:, :],
                    start=True, stop=True,
                )
                tot_sb = persist.tile([1, 1], f32, name="tot_sb")
                nc.vector.tensor_copy(tot_sb[:, :], tot_ps[:, :])
                nc.sync.dma_start(out_d[:, :], tot_sb[:, :])

    orig_to_json = nc.to_json_bytes
    nc.to_json_bytes = lambda: _patch_bir_bytes(orig_to_json())
    return nc


_NC = None


def _get_nc():
    global _NC
    if _NC is None:
        _NC = _build_bass()
    return _NC


def _pack_dr(a_t: np.ndarray) -> list[np.ndarray]:
    """[D, N] (already transposed, scaled, any float dtype) -> per-kpair
    fp8 DoubleRow operands [128, 2, N] with element [p, j, n] = a_t[kp*256 +
    j*128 + p, n]."""
    import ml_dtypes

    d, n = a_t.shape
    assert d == D
    a8 = np.asarray(a_t, ml_dtypes.float8_e4m3)
    a8 = a8.reshape(NKP, 2, P, n).transpose(0, 2, 1, 3)  # [kp, p, j, n]
    return [np.ascontiguousarray(a8[kp]) for kp in range(NKP)]


def _make_in_maps(soft_x, hard_x, targets, centers, weight):
    import ml_dtypes

    bf = ml_dtypes.bfloat16
    soft_x = np.asarray(soft_x, np.float32)
    hard_x = np.asarray(hard_x, np.float32)
    targets = np.asarray(targets)
    weight = np.asarray(weight, np.float32)
    centers = np.asarray(centers, np.float32)

    wt8 = _pack_dr(np.ascontiguousarray(weight.T) * FP8_SCALE)
    w_bf = np.ascontiguousarray(weight.astype(bf))
    cen_bf = np.ascontiguousarray(centers.astype(bf))

    in_maps = []
    for i in range(N_CORES):
        sl = slice(i * BS, (i + 1) * BS)
        X = np.concatenate([soft_x[sl], hard_x[sl]], axis=0)
        xt8 = _pack_dr(np.ascontiguousarray(X.T) * FP8_SCALE)
        tg = np.ascontiguousarray(targets[sl].astype(np.int32).reshape(BS, 1))
        in_maps.append({
            "xt0": xt8[0], "xt1": xt8[1], "wt0": wt8[0], "wt1": wt8[1],
            "x": np.ascontiguousarray(X.astype(bf)), "w": w_bf,
            "cen": cen_bf, "tgt": tg,
        })
    return in_maps


def _run(inputs, trace=False):
    from concourse.bass_utils import run_bass_kernel_spmd

    nc = _get_nc()
    in_maps = _make_in_maps(**inputs)
    res = run_bass_kernel_spmd(
        nc, in_maps, core_ids=list(range(N_CORES)), trace=trace
    )
    total = sum(float(r["out"][0, 0]) for r in res.results)
    return np.float32(total / B_FULL), res


def kernel(soft_x, hard_x, targets, centers, weight):
    loss, _ = _run(
        dict(soft_x=soft_x, hard_x=hard_x, targets=targets,
             centers=centers, weight=weight)
    )
    return loss


# revision 9
# speedup vs baseline: 1.0183x; 1.0183x over previous
"""DPQ joint classification loss on 8 Trainium2 NeuronCores.

reference math (B=4096, D=512, C=10000):
    soft_pred = soft_x @ weight.T ; hard_pred = hard_x @ weight.T
    loss = CE(soft_pred, t) + CE(hard_pred, t)
           + 0.5 * 0.5*(||soft_x - centers[t]||^2 + ||hard_x - centers[t]||^2) / B

Sharding: data-parallel over batch. Core i gets soft rows [i*512,(i+1)*512)
and the matching hard rows, stacked into X = [1024, 512]; weight/centers are
replicated. Each core returns one scalar:
    sum_rows( logsumexp(X @ W^T) - logit_at_target + 0.25*||X - centers[t]||^2 )
and the host computes loss = sum(cores) / B.

Per-core pipeline (engine balance: ACT ~70us is the pace-setter, PE fp8
~69us, DVE ~40us, Pool runs the SWDGE gathers):
  - PE: fp8(e4m3) DoubleRow GEMM at 2x rate: both operands are packed
    [128p, 2, free] so one matmul contracts 256 of the 512 k-dim. x and w
    are pre-scaled by 16 on the host (keeps w out of the fp8 subnormal
    range); the exp undoes the 256x logit scale via its input scale.
  - ACT: exp straight out of PSUM into a discarded bf16 tile (no
    max-subtraction: logits are ~N(0, 0.31), exp is safe in fp32). No
    accum_out: the 283ns/instr accumulator read would make ACT the
    bottleneck, so the row-sums ride on DVE instead.
  - DVE: per-tile row-sum of the bf16 exp tile via tensor_scalar+accum_out
    (4x mode), plus the exact bf16 target-logit (rowsum(x * w_gather)) and
    quantization (rowsum((x - c_gather)^2)) terms and the final combine.
  - GPSIMD: indirect-DMA row gathers weight[targets], centers[targets]
    from bf16 copies of the tables.
  - PE again: cross-partition sum via ones-matmul; DMA scalar out.
"""

import json

import numpy as np

B_FULL = 4096
D = 512
C = 10000
N_CORES = 8
BS = B_FULL // N_CORES          # 512 rows per core per tensor
B = 2 * BS                      # 1024 stacked rows per core
P = 128
NB = B // P                     # 8 row chunks
NKP = 2                         # k-pairs: 512 = 2 * (2*128)
GW = 2048                       # class-group width = 4 PSUM banks
PARAM = 0.5
FP8_SCALE = 16.0                # per-operand pre-scale before e4m3 cast


def _patch_bir_bytes(b: bytes, max_waits: int = 1) -> bytes:
    """Adapt Tile-emitted BIR to this walrus build: it supports only one
    sync-wait per instruction (excess waits move to preceding NoOps) and
    rejects the EVENT_SEMAPHORE_RANGE_CLEAR raw-ISA encoding (replaced by
    per-semaphore write-0 EventSemaphore ops)."""
    d = json.loads(b)
    for f in d["functions"]:
        for blk in f["blocks"]:
            new_insts = []
            for ins in blk["instructions"]:
                if (
                    ins.get("opcode") == "ISA"
                    and ins.get("op_name") == "EVENT_SEMAPHORE_RANGE_CLEAR"
                ):
                    ad = ins.get("ant_dict") or {}
                    for sem_id in range(ad["range_first"], ad["range_last"] + 1):
                        new_insts.append({
                            "name": f"{ins['name']}_clr{sem_id}",
                            "opcode": "EventSemaphore",
                            "engine": ins["engine"],
                            "ins": [],
                            "outs": [],
                            "debug": ins.get("debug"),
                            "sync_info": {
                                "on_wait": [],
                                "on_update": [{
                                    "ant_name": f"semclr_{sem_id}",
                                    "id": sem_id,
                                    "sync_type": "semaphore",
                                    "update_mode": "sem-wr-imm",
                                    "update_value": 0,
                                }],
                            },
                        })
                    continue
                si = ins.get("sync_info")
                waits = (si or {}).get("on_wait") or []
                if len(waits) > max_waits:
                    extra, keep = waits[:-max_waits], waits[-max_waits:]
                    idx = 0
                    while extra:
                        chunk, extra = extra[:max_waits], extra[max_waits:]
                        new_insts.append({
                            "name": f"{ins['name']}_w{idx}",
                            "opcode": "NoOp",
                            "engine": ins["engine"],
                            "ins": [],
                            "outs": [],
                            "debug": ins.get("debug"),
                            "sync_info": {"on_wait": chunk, "on_update": []},
                        })
                        idx += 1
                    si["on_wait"] = keep
                new_insts.append(ins)
            blk["instructions"] = new_insts
    return json.dumps(d).encode()


def _build_bass():
    import concourse.bass as bass
    import concourse.tile as tile
    from concourse import mybir

    f32 = mybir.dt.float32
    bf16 = mybir.dt.bfloat16
    f8 = mybir.dt.float8e4
    i32 = mybir.dt.int32
    AF = mybir.ActivationFunctionType
    OP = mybir.AluOpType
    DR = mybir.MatmulPerfMode.DoubleRow

    # first groups are narrow so the first psum bank (and ACT) fires while
    # the bulk of the weight stream is still in flight
    widths = [512, 1536, 2048, 2048, 2048, 1808]
    assert sum(widths) == C
    groups = []
    c0 = 0
    for w in widths:
        groups.append((c0, w))
        c0 += w
    NG = len(groups)

    nc = bass.Bass()
    # fp8 DoubleRow operands, packed [128p, j=2, free]; element [p, j, n]
    # holds the k-index kp*256 + j*128 + p.
    xt_d = [nc.dram_tensor(f"xt{kp}", [P, 2, B], f8, kind="ExternalInput")
            for kp in range(NKP)]
    wt_d = [nc.dram_tensor(f"wt{kp}", [P, 2, C], f8, kind="ExternalInput")
            for kp in range(NKP)]
    x_d = nc.dram_tensor("x", [B, D], bf16, kind="ExternalInput")
    w_d = nc.dram_tensor("w", [C, D], bf16, kind="ExternalInput")
    cen_d = nc.dram_tensor("cen", [C, D], bf16, kind="ExternalInput")
    tgt_d = nc.dram_tensor("tgt", [BS, 1], i32, kind="ExternalInput")
    out_d = nc.dram_tensor("out", [1, 1], f32, kind="ExternalOutput")

    with tile.TileContext(nc) as tc:
        with (
            tc.tile_pool(name="persist", bufs=1) as persist,
            tc.tile_pool(name="wtp", bufs=2) as wtp,
            tc.tile_pool(name="scratch", bufs=3) as scratch,
        ):
            # ---- resident loads ----
            # xt feeds the matmuls: issue on the ACT ring (idle at startup)
            # so the sync ring's first trigger is already the first wt group.
            xt_sb = []
            for kp in range(NKP):
                t = persist.tile([P, 2, B], f8, tag=f"xt{kp}", name=f"xt{kp}")
                nc.scalar.dma_start(t[:, :, :], xt_d[kp][:, :, :])
                xt_sb.append(t)
            # x/tgt feed only the (small) DVE aux path; gpsimd SWDGE keeps
            # them off the HWDGE queues that stream wt.
            tgt_sb = []
            for c in range(BS // P):
                t = persist.tile([P, 1], i32, tag=f"tgt{c}", name=f"tgt{c}")
                nc.gpsimd.dma_start(t[:, :], tgt_d[c * P:(c + 1) * P, :])
                tgt_sb.append(t)
            x_sb = []
            for b in range(NB):
                t = persist.tile([P, D], bf16, tag=f"x{b}", name=f"x{b}")
                nc.gpsimd.dma_start(t[:, :], x_d[b * P:(b + 1) * P, :])
                x_sb.append(t)

            # ---- gathers: weight[targets], centers[targets] (bf16) ----
            wg_sb, cg_sb = [], []
            for c in range(BS // P):
                wg = persist.tile([P, D], bf16, tag=f"wg{c}", name=f"wg{c}")
                nc.gpsimd.indirect_dma_start(
                    out=wg[:, :], out_offset=None, in_=w_d[:, :],
                    in_offset=bass.IndirectOffsetOnAxis(ap=tgt_sb[c][:, :1], axis=0),
                )
                wg_sb.append(wg)
                cg = persist.tile([P, D], bf16, tag=f"cg{c}", name=f"cg{c}")
                nc.gpsimd.indirect_dma_start(
                    out=cg[:, :], out_offset=None, in_=cen_d[:, :],
                    in_offset=bass.IndirectOffsetOnAxis(ap=tgt_sb[c][:, :1], axis=0),
                )
                cg_sb.append(cg)

            # ---- small result tiles ----
            sums = persist.tile([P, NB * NG], f32, name="sums")
            se = persist.tile([P, NB], f32, name="se")
            lse = persist.tile([P, NB], f32, name="lse")
            tcol = persist.tile([P, NB], f32, name="tcol")
            qcol = persist.tile([P, NB], f32, name="qcol")
            ctr1 = persist.tile([P, NB], f32, name="ctr1")
            ctr2 = persist.tile([P, NB], f32, name="ctr2")
            rowtot = persist.tile([P, 1], f32, name="rowtot")
            ones = persist.tile([P, 1], f32, name="ones")
            nc.vector.memset(ones[:, :], 1.0)

            # ---- aux path on DVE (all bf16 for the 2x/4x modes): target
            # logits + quantization ----
            for b in range(NB):
                c = b % (BS // P)
                pr = scratch.tile([P, D], bf16, tag="pr", name=f"pr{b}")
                nc.vector.tensor_mul(pr[:, :], x_sb[b][:, :], wg_sb[c][:, :])
                dm0 = scratch.tile([P, D], bf16, tag="dm0", name=f"dm0_{b}")
                nc.vector.tensor_scalar(
                    out=dm0[:, :], in0=pr[:, :], scalar1=1.0, scalar2=0.0,
                    op0=OP.mult, op1=OP.add, accum_out=tcol[:, b:b + 1],
                )
                df = scratch.tile([P, D], bf16, tag="df", name=f"df{b}")
                nc.vector.tensor_sub(df[:, :], x_sb[b][:, :], cg_sb[c][:, :])
                sq = scratch.tile([P, D], bf16, tag="sq", name=f"sq{b}")
                nc.vector.tensor_mul(sq[:, :], df[:, :], df[:, :])
                dm1 = scratch.tile([P, D], bf16, tag="dm1", name=f"dm1_{b}")
                nc.vector.tensor_scalar(
                    out=dm1[:, :], in0=sq[:, :], scalar1=1.0, scalar2=0.0,
                    op0=OP.mult, op1=OP.add, accum_out=qcol[:, b:b + 1],
                )

            # ---- main GEMM (fp8 DoubleRow) + exp, row-sums on DVE ----
            with tc.tile_pool(name="psum", bufs=2, space="PSUM") as psum_pool:
                for g, (c0, cw) in enumerate(groups):
                    wt_g = []
                    for kp in range(NKP):
                        t = wtp.tile([P, 2, cw], f8, tag=f"wt{kp}", name=f"wt{kp}_{g}")
                        nc.sync.dma_start(t[:, :, :cw], wt_d[kp][:, :, c0:c0 + cw])
                        wt_g.append(t)
                    for b in range(NB):
                        ps = psum_pool.tile([P, cw], f32, tag="ps", name=f"ps{g}_{b}")
                        nbank = (cw + 511) // 512
                        for bank in range(nbank):
                            s0 = bank * 512
                            sw = min(512, cw - s0)
                            for kp in range(NKP):
                                nc.tensor.matmul(
                                    ps[:, s0:s0 + sw],
                                    lhsT=xt_sb[kp][:, :, b * P:(b + 1) * P],
                                    rhs=wt_g[kp][:, :, s0:s0 + sw],
                                    start=(kp == 0), stop=(kp == NKP - 1),
                                    perf_mode=DR,
                                )
                        es = scratch.tile([P, cw], f32, tag="es", name=f"es{g}_{b}")
                        nc.scalar.activation(
                            es[:, :cw], ps[:, :cw], AF.Exp,
                            scale=1.0 / (FP8_SCALE * FP8_SCALE),
                            accum_out=sums[:, b * NG + g: b * NG + g + 1],
                        )

            # ---- logsumexp + per-row combine ----
            for b in range(NB):
                nc.vector.tensor_reduce(
                    out=se[:, b:b + 1], in_=sums[:, b * NG:(b + 1) * NG],
                    axis=mybir.AxisListType.X, op=OP.add,
                )
            nc.scalar.activation(lse[:, :], se[:, :], AF.Ln)
            nc.vector.tensor_sub(ctr1[:, :], lse[:, :], tcol[:, :])
            nc.vector.scalar_tensor_tensor(
                out=ctr2[:, :], in0=qcol[:, :], scalar=0.25, in1=ctr1[:, :],
                op0=OP.mult, op1=OP.add,
            )
            nc.vector.tensor_reduce(
                out=rowtot[:, :], in_=ctr2[:, :],
                axis=mybir.AxisListType.X, op=OP.add,
            )

            # ---- cross-partition sum via ones-matmul, write scalar ----
            with tc.tile_pool(name="psum2", bufs=1, space="PSUM") as pp2:
                tot_ps = pp2.tile([1, 1], f32, name="tot_ps")
                nc.tensor.matmul(
                    tot_ps[:, :], lhsT=rowtot[:, :], rhs=ones[:, :],
                    start=True, stop=True,
                )
                tot_sb = persist.tile([1, 1], f32, name="tot_sb")
                nc.vector.tensor_copy(tot_sb[:, :], tot_ps[:, :])
                nc.sync.dma_start(out_d[:, :], tot_sb[:, :])

    orig_to_json = nc.to_json_bytes
    nc.to_json_bytes = lambda: _patch_bir_bytes(orig_to_json())
    return nc


_NC = None


def _get_nc():
    global _NC
    if _NC is None:
        _NC = _build_bass()
    return _NC


def _pack_dr(a_t: np.ndarray) -> list[np.ndarray]:
    """[D, N] (already transposed, scaled, any float dtype) -> per-kpair
    fp8 DoubleRow operands [128, 2, N] with element [p, j, n] = a_t[kp*256 +
    j*128 + p, n]."""
    import ml_dtypes

    d, n = a_t.shape
    assert d == D
    a8 = np.asarray(a_t, ml_dtypes.float8_e4m3)
    a8 = a8.reshape(NKP, 2, P, n).transpose(0, 2, 1, 3)  # [kp, p, j, n]
    return [np.ascontiguousarray(a8[kp]) for kp in range(NKP)]


def _make_in_maps(soft_x, hard_x, targets, centers, weight):
    import ml_dtypes

    bf = ml_dtypes.bfloat16
    soft_x = np.asarray(soft_x, np.float32)
    hard_x = np.asarray(hard_x, np.float32)
    targets = np.asarray(targets)
    weight = np.asarray(weight, np.float32)
    centers = np.asarray(centers, np.float32)

    wt8 = _pack_dr(np.ascontiguousarray(weight.T) * FP8_SCALE)
    w_bf = np.ascontiguousarray(weight.astype(bf))
    cen_bf = np.ascontiguousarray(centers.astype(bf))

    in_maps = []
    for i in range(N_CORES):
        sl = slice(i * BS, (i + 1) * BS)
        X = np.concatenate([soft_x[sl], hard_x[sl]], axis=0)
        xt8 = _pack_dr(np.ascontiguousarray(X.T) * FP8_SCALE)
        tg = np.ascontiguousarray(targets[sl].astype(np.int32).reshape(BS, 1))
        in_maps.append({
            "xt0": xt8[0], "xt1": xt8[1], "wt0": wt8[0], "wt1": wt8[1],
            "x": np.ascontiguousarray(X.astype(bf)), "w": w_bf,
            "cen": cen_bf, "tgt": tg,
        })
    return in_maps


def _run(inputs, trace=False):
    from concourse.bass_utils import run_bass_kernel_spmd

    nc = _get_nc()
    in_maps = _make_in_maps(**inputs)
    res = run_bass_kernel_spmd(
        nc, in_maps, core_ids=list(range(N_CORES)), trace=trace
    )
    total = sum(float(r["out"][0, 0]) for r in res.results)
    return np.float32(total / B_FULL), res


def kernel(soft_x, hard_x, targets, centers, weight):
    loss, _ = _run(
        dict(soft_x=soft_x, hard_x=hard_x, targets=targets,
             centers=centers, weight=weight)
    )
    return loss
